# revision 1
# baseline (speedup 1.0000x reference)
"""MLA (Multi-head Latent Attention) Bass/Tile kernel for 8 Trainium2 NeuronCores.

Problem: nn_MultiHeadLatentAttention_81707457839331
  B=2, S=2048, HID=2048, NH=16 heads, NOPE=128, ROPE=64, VD=128, QKD=192,
  KVR=512, QR=1536, fp32.

Sharding (single NEFF, SPMD on 8 cores):
  core c -> batch b = c//4, head group g = c%4 (4 heads each).
  Down-projections (q_down, kv_down) are replicated within each 4-core batch
  group (per the sharding hint); q_up/kv_up/attention/o_proj are head-sharded.
  Each core emits a partial o_proj output [S, HID]; the host sums the 4
  partials per batch (unshard step).

On-device layout strategy: everything is kept in [feature, token] ("T")
layouts so no on-device transposes are needed anywhere:
  - matmuls feed each other directly (contraction dim on partitions),
  - RMSNorm per-token scales are folded: gamma into the up-weights (host),
    1/rms(q_lat) into the qT psum eviction, 1/rms(c_kv) into c_kvT,
  - RoPE rotate_half is folded into extra host-rotated weight columns, so
    on-device RoPE is a pure elementwise  x*cos + x_rot*sin,
  - the shared roped key is materialized twice with zero halves
    ([k_rope;0] / [0;k_rope]) so per-head rope score matmuls run at full
    K=128 with base-partition 0 (no LDW serialization),
  - softmax runs unnormalized (scores are bounded); prob sums are
    accumulated on DVE and reduced across partitions on GpSimd
    (partition_all_reduce), keeping TensorE free of m=1 matmuls,
  - causal masking skips strictly-upper score tiles and adds -1e30 masks
    (4 precomputed patterns) on diagonal tiles.
All matmul inputs are float32r (full PE rate at moving dim 512).
"""

import numpy as np

import concourse.bass as bass
import concourse.bass_isa as bass_isa
import concourse.mybir as mybir
import concourse.tile as tile
from concourse import bacc
from concourse.bass import ds, ts
from concourse.bass_utils import run_bass_kernel_spmd

F32 = mybir.dt.float32
F32R = mybir.dt.float32r
AF = mybir.ActivationFunctionType

B, S, HID, NH = 2, 2048, 2048, 16
NOPE, ROPE, VD = 128, 64, 128
QKD = NOPE + ROPE
KVR, QR = 512, 1536
EPS = 1e-6
SCALE = QKD ** (-0.5)
P = 128

NHC = HID // P            # 16 hidden chunks
NQC = QR // P             # 12 q-latent chunks
NFC = 18                  # down-proj output chunks (12 qlat + 4 ckv + rope + rot)
NKC = KVR // P            # 4 ckv chunks
NTT = S // 512            # 4 token tiles of 512
NTC = S // P              # 16 token chunks of 128
NDQ = 8                   # q_up output chunks: 4 nope + 2 rope-pairs + 2 rot-pairs
NEG = -1e30


def _emit(tc):
    nc = tc.nc
    hid_in = nc.dram_tensor("hid", [P, NHC, S], F32R, kind="ExternalInput").ap()
    cos_in = nc.dram_tensor("cos2", [P, S], F32, kind="ExternalInput").ap()
    sin_in = nc.dram_tensor("sin2", [P, S], F32, kind="ExternalInput").ap()
    wd_in = nc.dram_tensor("wd", [NFC, P, NHC, P], F32R, kind="ExternalInput").ap()
    wqup_in = nc.dram_tensor("wqup", [P, NQC, NDQ * P], F32R, kind="ExternalInput").ap()
    wkup_in = nc.dram_tensor("wkup", [P, NKC, 512], F32R, kind="ExternalInput").ap()
    wvup_in = nc.dram_tensor("wvup", [P, NKC, 512], F32R, kind="ExternalInput").ap()
    wo_in = nc.dram_tensor("wo", [P, 4, HID], F32R, kind="ExternalInput").ap()
    out_d = nc.dram_tensor("out", [S, HID], F32, kind="ExternalOutput").ap()

    with (
        tc.tile_pool(name="const", bufs=1) as constp,
        tc.tile_pool(name="dram", bufs=1, space="DRAM") as dramp,
    ):
        eps_kv = constp.tile([P, 1], F32)
        nc.vector.memset(eps_kv, EPS)
        eps_q = constp.tile([P, 1], F32)
        nc.vector.memset(eps_q, EPS / (SCALE * SCALE))
        # 4 causal additive masks: mask_k[p, x] = 0 if x - p - 128k >= 0 else -1e30
        masks = []
        for k in range(4):
            m = constp.tile([P, 512], F32, name=f"mask{k}")
            nc.gpsimd.memset(m, 0.0)
            nc.gpsimd.affine_select(
                out=m, in_=m, pattern=[[1, 512]],
                compare_op=mybir.AluOpType.is_ge, fill=NEG,
                base=-128 * k, channel_multiplier=-1,
            )
            masks.append(m)

        # DRAM scratch, laid out so every reload is partition-contiguous
        latq = dramp.tile([NTT, P, NQC, 512], F32R)   # q-latent, token-tile major
        latkv = dramp.tile([P, 6, S], F32R)           # ckv(4) + rope-dup + rot-dup
        kt_d = dramp.tile([P, 4, S], F32R)            # 4 heads' k_nope.T
        krope_d = dramp.tile([P, S], F32R)            # [k_roped; 0]
        kropo_d = dramp.tile([P, S], F32R)            # [0; k_roped]
        v_d = dramp.tile([P, NTC, 512], F32R)         # V in [token, 4*VD]
        o_d = dramp.tile([P, 4, S], F32R)             # attention out, head-major

        # ---------------- Phase A: fused down-projection ----------------
        # (kv chunks first so phase B overlaps the q-latent half)
        with (
            tc.tile_pool(name="pa_hid", bufs=1) as ph,
            tc.tile_pool(name="pa_w", bufs=2) as pw,
            tc.tile_pool(name="pa_row", bufs=2) as prow,
            tc.tile_pool(name="pa_ps", bufs=4, space="PSUM") as pps,
        ):
            with nc.named_scope("phaseA"):
                hid_sb = ph.tile([P, NHC, S], F32R)
                for hc in range(NHC):
                    nc.sync.dma_start(hid_sb[:, hc, :], hid_in[:, hc, :])
                for fc in list(range(12, NFC)) + list(range(12)):
                    w_sb = pw.tile([P, NHC, P], F32R, name="wslice")
                    nc.sync.dma_start(w_sb, wd_in[fc])
                    row = prow.tile([P, S], F32R, name="arow")
                    for tt in range(NTT):
                        ps = pps.tile([P, 512], F32, name="aps")
                        for hc in range(NHC):
                            nc.tensor.matmul(
                                ps, w_sb[:, hc, :], hid_sb[:, hc, ts(tt, 512)],
                                start=(hc == 0), stop=(hc == NHC - 1),
                            )
                        nc.vector.tensor_copy(row[:, ts(tt, 512)], ps)
                    if fc >= 12:
                        nc.sync.dma_start(latkv[:, fc - 12, :], row)
                    else:
                        for tt in range(NTT):
                            nc.sync.dma_start(latq[tt][:, fc, :],
                                              row[:, ts(tt, 512)])

        # ---------------- Phase B: kv norm + rope-k + kv_up ----------------
        with (
            tc.tile_pool(name="pb", bufs=1) as pb,
            tc.tile_pool(name="pb_tmp", bufs=2) as pbt,
            tc.tile_pool(name="pb_row", bufs=2) as pbr,
            tc.tile_pool(name="pb_ps", bufs=3, space="PSUM") as pps2,
        ):
            with nc.named_scope("phaseB"):
                kv_sb = pb.tile([P, 6, S], F32R)
                nc.sync.dma_start(kv_sb, latkv)
                cos2b = pb.tile([P, S], F32)
                sin2b = pb.tile([P, S], F32)
                nc.sync.dma_start(cos2b, cos_in)
                nc.sync.dma_start(sin2b, sin_in)
                # rkv = 1/sqrt(mean(ckv^2) + eps), computed on all partitions
                rkv_b = pb.tile([P, S], F32)
                for tt in range(NTT):
                    acc = pbt.tile([P, 512], F32, name="bacc")
                    nc.scalar.square(acc, kv_sb[:, 0, ts(tt, 512)])
                    for fc in range(1, NKC):
                        sq = pbt.tile([P, 512], F32, name="bsq")
                        nc.scalar.square(sq, kv_sb[:, fc, ts(tt, 512)])
                        nc.vector.tensor_add(acc, acc, sq)
                    ar = pbt.tile([P, 512], F32, name="bar")
                    nc.gpsimd.partition_all_reduce(ar, acc, channels=P,
                                                   reduce_op=bass_isa.ReduceOp.add)
                    nc.scalar.activation(ar, ar, AF.Sqrt, bias=eps_kv,
                                         scale=1.0 / KVR)
                    nc.vector.reciprocal(rkv_b[:, ts(tt, 512)], ar)
                for fc in range(NKC):
                    nc.vector.tensor_mul(kv_sb[:, fc, :], kv_sb[:, fc, :], rkv_b)
                # roped shared key; then split into [roped;0] and [0;roped]
                krd = pb.tile([P, S], F32R)
                t1 = pbt.tile([P, S], F32, name="bt1")
                nc.vector.tensor_mul(t1, kv_sb[:, 4, :], cos2b)
                nc.vector.tensor_mul(krd, kv_sb[:, 5, :], sin2b)
                nc.vector.tensor_add(krd, krd, t1)
                kre = pb.tile([P, S], F32R)
                kro = pb.tile([P, S], F32R)
                nc.vector.tensor_copy(kre, krd)
                nc.vector.tensor_scalar_mul(kre[64:128, :], kre[64:128, :], 0.0)
                nc.vector.tensor_copy(kro, krd)
                nc.vector.tensor_scalar_mul(kro[0:64, :], kro[0:64, :], 0.0)
                nc.sync.dma_start(krope_d, kre)
                nc.sync.dma_start(kropo_d, kro)
                # kv_up
                wk_sb = pb.tile([P, NKC, 512], F32R)
                wv_sb = pb.tile([P, NKC, 512], F32R)
                nc.sync.dma_start(wk_sb, wkup_in)
                nc.sync.dma_start(wv_sb, wvup_in)
                for d in range(4):
                    krow = pbr.tile([P, S], F32R, name="krow")
                    for tt in range(NTT):
                        ps = pps2.tile([P, 512], F32, name="bps")
                        for fc in range(NKC):
                            nc.tensor.matmul(
                                ps, wk_sb[:, fc, ds(d * P, P)],
                                kv_sb[:, fc, ts(tt, 512)],
                                start=(fc == 0), stop=(fc == NKC - 1),
                            )
                        nc.vector.tensor_copy(krow[:, ts(tt, 512)], ps)
                    nc.sync.dma_start(kt_d[:, d, :], krow)
                for tch in range(NTC):
                    ps = pps2.tile([P, 512], F32, name="bpsv")
                    for fc in range(NKC):
                        nc.tensor.matmul(
                            ps, kv_sb[:, fc, ds(tch * P, P)], wv_sb[:, fc, :],
                            start=(fc == 0), stop=(fc == NKC - 1),
                        )
                    vrow = pbr.tile([P, 512], F32R, name="vrow")
                    nc.vector.tensor_copy(vrow, ps)
                    nc.sync.dma_start(v_d[:, tch, :], vrow)

        # qT lives in SBUF from phase C through phase D
        if True:
            with tc.tile_pool(name="pq", bufs=1) as pq:
                qT = pq.tile([P, 6, S], F32R)  # 4 nope + 2 roped pairs

                # ---------------- Phase C: q_up + rope-q + q-norm ----------------
                with (
                    tc.tile_pool(name="pc_w", bufs=1) as pcw,
                    tc.tile_pool(name="pc_slab", bufs=2) as pcs,
                    tc.tile_pool(name="pc_tmp", bufs=3) as pct,
                    tc.tile_pool(name="pc_ps", bufs=4, space="PSUM") as pps3,
                ):
                    with nc.named_scope("phaseC"):
                        wq_sb = pcw.tile([P, NQC, NDQ * P], F32R)
                        nc.sync.dma_start(wq_sb, wqup_in)
                        cos2c = pcw.tile([P, S], F32)
                        sin2c = pcw.tile([P, S], F32)
                        nc.sync.dma_start(cos2c, cos_in)
                        nc.sync.dma_start(sin2c, sin_in)
                        for tt in range(NTT):
                            slab = pcs.tile([P, NQC, 512], F32R, name="qslabin")
                            nc.sync.dma_start(slab, latq[tt])
                            # rq = SCALE/sqrt(mean(qlat^2) + eps), all partitions
                            acc = pct.tile([P, 512], F32, name="cacc")
                            nc.scalar.square(acc, slab[:, 0, :])
                            for fc in range(1, NQC):
                                sq = pct.tile([P, 512], F32, name="csq")
                                nc.scalar.square(sq, slab[:, fc, :])
                                nc.vector.tensor_add(acc, acc, sq)
                            rq_b = pct.tile([P, 512], F32, name="crqb")
                            nc.gpsimd.partition_all_reduce(
                                rq_b, acc, channels=P,
                                reduce_op=bass_isa.ReduceOp.add)
                            nc.scalar.activation(rq_b, rq_b, AF.Sqrt, bias=eps_q,
                                                 scale=1.0 / (QR * SCALE * SCALE))
                            nc.vector.reciprocal(rq_b, rq_b)
                            rope_ps = []
                            for d in range(NDQ):
                                ps = pps3.tile([P, 512], F32, name="cps")
                                for fc in range(NQC):
                                    nc.tensor.matmul(
                                        ps, wq_sb[:, fc, ds(d * P, P)],
                                        slab[:, fc, :],
                                        start=(fc == 0), stop=(fc == NQC - 1),
                                    )
                                if d < 4:
                                    # fused eviction: qT = psum * rq
                                    nc.vector.tensor_mul(
                                        qT[:, d, ts(tt, 512)], ps, rq_b)
                                else:
                                    rope_ps.append(ps)
                            for pr in range(2):
                                t1 = pct.tile([P, 512], F32, name="ct1")
                                t2 = pct.tile([P, 512], F32, name="ct2")
                                nc.vector.tensor_mul(t1, rope_ps[pr],
                                                     cos2c[:, ts(tt, 512)])
                                nc.vector.tensor_mul(t2, rope_ps[2 + pr],
                                                     sin2c[:, ts(tt, 512)])
                                nc.vector.tensor_add(t1, t1, t2)
                                nc.vector.tensor_mul(
                                    qT[:, 4 + pr, ts(tt, 512)], t1, rq_b)

                # ---------------- Phase D: attention ----------------
                with (
                    tc.tile_pool(name="pd_kv", bufs=1) as pdkv,
                    tc.tile_pool(name="pd_e", bufs=4) as pde,
                    tc.tile_pool(name="pd_t", bufs=3) as pdt,
                    tc.tile_pool(name="pd_psc", bufs=3, space="PSUM") as pdsc,
                    tc.tile_pool(name="pd_pso", bufs=2, space="PSUM") as pdo,
                ):
                    with nc.named_scope("phaseD"):
                        kt_sb = pdkv.tile([P, 4, S], F32R)
                        nc.sync.dma_start(kt_sb, kt_d)
                        kre_sb = pdkv.tile([P, S], F32R)
                        nc.sync.dma_start(kre_sb, krope_d)
                        kro_sb = pdkv.tile([P, S], F32R)
                        nc.sync.dma_start(kro_sb, kropo_d)
                        v_sb = pdkv.tile([P, NTC, 512], F32R)
                        nc.sync.dma_start(v_sb, v_d)

                        for h in range(4):
                            krop = kre_sb if h % 2 == 0 else kro_sb
                            qp = qT[:, 4 + h // 2, :]
                            for i in range(NTT):
                                ps_o = pdo.tile([P, 512], F32, name="pso")
                                eacc = pdt.tile([P, 512], F32, name="eacc")
                                jmax = 4 * i + 3
                                for jc in range(jmax + 1):
                                    ps_sc = pdsc.tile([P, 512], F32, name="psc")
                                    nc.tensor.matmul(
                                        ps_sc, kt_sb[:, h, ds(jc * P, P)],
                                        qT[:, h, ts(i, 512)],
                                        start=True, stop=False)
                                    nc.tensor.matmul(
                                        ps_sc, krop[:, ds(jc * P, P)],
                                        qp[:, ts(i, 512)],
                                        start=False, stop=True)
                                    if jc >= 4 * i:
                                        nc.vector.tensor_add(ps_sc, ps_sc,
                                                             masks[jc - 4 * i])
                                    et = pde.tile([P, 512], F32R, name="et")
                                    nc.scalar.activation(et, ps_sc, AF.Exp)
                                    nc.tensor.matmul(
                                        ps_o, v_sb[:, jc, ds(h * P, P)], et,
                                        start=(jc == 0), stop=(jc == jmax))
                                    if jc == 0:
                                        nc.vector.tensor_copy(eacc, et)
                                    else:
                                        nc.vector.tensor_add(eacc, eacc, et)
                                ar = pdt.tile([P, 512], F32, name="dar")
                                nc.gpsimd.partition_all_reduce(
                                    ar, eacc, channels=P,
                                    reduce_op=bass_isa.ReduceOp.add)
                                nc.vector.reciprocal(ar, ar)
                                ost = pdt.tile([P, 512], F32R, name="ost")
                                nc.vector.tensor_mul(ost, ps_o, ar)
                                nc.sync.dma_start(o_d[:, h, ts(i, 512)], ost)

        # ---------------- Phase F: o_proj partial ----------------
        with (
            tc.tile_pool(name="pf_w", bufs=1) as pfw,
            tc.tile_pool(name="pf_row", bufs=2) as pfr,
            tc.tile_pool(name="pf_ps", bufs=4, space="PSUM") as pfp,
        ):
            with nc.named_scope("phaseF"):
                wo_sb = pfw.tile([P, 4, HID], F32R)
                nc.sync.dma_start(wo_sb, wo_in)
                outHT = pfw.tile([P, 4, S], F32R)
                nc.sync.dma_start(outHT, o_d)
                for tch in range(NTC):
                    orow = pfr.tile([P, HID], F32, name="orow")
                    for ct in range(4):
                        ps = pfp.tile([P, 512], F32, name="fps")
                        for hh in range(4):
                            nc.tensor.matmul(
                                ps, outHT[:, hh, ds(tch * P, P)],
                                wo_sb[:, hh, ts(ct, 512)],
                                start=(hh == 0), stop=(hh == 3),
                            )
                        nc.vector.tensor_copy(orow[:, ts(ct, 512)], ps)
                    nc.sync.dma_start(out_d[ds(tch * P, P), :], orow)


_NC_CACHE = None


def _build_nc():
    global _NC_CACHE
    if _NC_CACHE is None:
        nc = bacc.Bacc("TRN2", target_bir_lowering=False, debug=False,
                       num_devices=8)
        with tile.TileContext(nc) as tc:
            _emit(tc)
        nc.compile()
        _NC_CACHE = nc
    return _NC_CACHE


def _shard_inputs(hidden_states, cos, sin, Wq_down, q_gamma, Wq_up,
                  Wkv_down, kv_gamma, Wkv_up, Wo):
    f32 = np.float32
    hid = np.ascontiguousarray(np.asarray(hidden_states, dtype=f32))
    cos = np.asarray(cos, dtype=f32)
    sin = np.asarray(sin, dtype=f32)
    Wqd = np.asarray(Wq_down, dtype=f32)
    Wkd = np.asarray(Wkv_down, dtype=f32)
    qg = np.asarray(q_gamma, dtype=f32)
    kvg = np.asarray(kv_gamma, dtype=f32)
    Wqu = np.asarray(Wq_up, dtype=f32) * qg[None, :]
    Wku = np.asarray(Wkv_up, dtype=f32) * kvg[None, :]
    Wo = np.asarray(Wo, dtype=f32)

    # shared: combined down-proj weight with host-rotated rope columns
    WqdT = Wqd.T                                   # [HID, QR]
    WckvT = Wkd[:KVR].T                            # [HID, KVR]
    krope = Wkd[KVR:].T                            # [HID, 64]
    krot = np.concatenate([-krope[:, 32:], krope[:, :32]], 1)
    WdT = np.concatenate([WqdT, WckvT, krope, krope, krot, krot], 1)  # [HID, 2304]
    wd = np.ascontiguousarray(
        WdT.reshape(NHC, P, NFC, P).transpose(2, 1, 0, 3))  # [18, 128, 16, 128]

    per_batch = []
    for b in range(B):
        h_sw = np.ascontiguousarray(
            hid[b].T.reshape(NHC, P, S).transpose(1, 0, 2))  # [128, 16, 2048]
        cT = cos[b].T                               # [64, S]
        sT = sin[b].T
        cos2 = np.ascontiguousarray(np.concatenate([cT, cT], 0))
        sin2 = np.ascontiguousarray(np.concatenate([sT, sT], 0))
        per_batch.append((h_sw, cos2, sin2))

    per_group = []
    for g in range(4):
        bn, br, brot = [], [], []
        for hl in range(4):
            h = 4 * g + hl
            blk = Wqu[h * QKD:(h + 1) * QKD]       # [192, QR]
            bn.append(blk[:NOPE])
            rr = blk[NOPE:]
            br.append(rr)
            brot.append(np.concatenate([-rr[32:], rr[:32]], 0))
        cols = bn + [np.concatenate([br[0], br[1]], 0),
                     np.concatenate([br[2], br[3]], 0),
                     np.concatenate([brot[0], brot[1]], 0),
                     np.concatenate([brot[2], brot[3]], 0)]
        WquT = np.concatenate(cols, 0).T           # [QR, 1024]
        wqup = np.ascontiguousarray(
            WquT.reshape(NQC, P, NDQ * P).transpose(1, 0, 2))  # [128, 12, 1024]
        kb, vb = [], []
        for hl in range(4):
            h = 4 * g + hl
            blk = Wku[h * (NOPE + VD):(h + 1) * (NOPE + VD)]
            kb.append(blk[:NOPE])
            vb.append(blk[NOPE:])
        WkuT = np.concatenate(kb, 0).T             # [KVR, 512]
        WvuT = np.concatenate(vb, 0).T
        wkup = np.ascontiguousarray(WkuT.reshape(NKC, P, 512).transpose(1, 0, 2))
        wvup = np.ascontiguousarray(WvuT.reshape(NKC, P, 512).transpose(1, 0, 2))
        WoT = Wo[:, g * 512:(g + 1) * 512].T       # [512, HID]
        wo = np.ascontiguousarray(WoT.reshape(4, P, HID).transpose(1, 0, 2))
        per_group.append((wqup, wkup, wvup, wo))

    in_maps = []
    for c in range(8):
        b, g = c // 4, c % 4
        h_sw, cos2, sin2 = per_batch[b]
        wqup, wkup, wvup, wo = per_group[g]
        in_maps.append({
            "hid": h_sw, "cos2": cos2, "sin2": sin2, "wd": wd,
            "wqup": wqup, "wkup": wkup, "wvup": wvup, "wo": wo,
        })
    return in_maps


def kernel(hidden_states, cos, sin, Wq_down, q_gamma, Wq_up,
           Wkv_down, kv_gamma, Wkv_up, Wo, _trace=False):
    nc = _build_nc()
    in_maps = _shard_inputs(hidden_states, cos, sin, Wq_down, q_gamma, Wq_up,
                            Wkv_down, kv_gamma, Wkv_up, Wo)
    res = run_bass_kernel_spmd(nc, in_maps, core_ids=list(range(8)),
                               trace=_trace)
    out = np.zeros((B, S, HID), dtype=np.float32)
    for c in range(8):
        out[c // 4] += res.results[c]["out"]
    if _trace:
        kernel.last_results = res
    return out



# revision 14
# speedup vs baseline: 1.6775x; 1.6775x over previous
"""MLA (Multi-head Latent Attention) Bass/Tile kernel for 8 Trainium2 NeuronCores.

Problem: nn_MultiHeadLatentAttention_81707457839331
  B=2, S=2048, HID=2048, NH=16 heads, NOPE=128, ROPE=64, VD=128, QKD=192,
  KVR=512, QR=1536, fp32 in/out.

Sharding (single NEFF, SPMD on 8 cores, collectives inside):
  core c -> batch b = c//4, head group g = c%4 (4 heads), token quarter
  tq = c%4 (512 tokens) for the down-projection.
  Phase A computes the fused down-projection ONLY for the core's token
  quarter, RMS-normalizes q-latent / c_kv and ropes the shared key locally,
  then two AllGathers (ckv+krd first, q-latent second) over the 4-core batch
  group replicate the full-sequence latents. kv_up/q_up/attention/o_proj are
  head-sharded as before; each core emits a partial o_proj [S, HID] and the
  host sums the 4 partials per batch.

Numerics/layout:
  - All matmul inputs are bf16 (PE rate is 1 cycle/row either way; bf16
    halves DMA, SBUF and DVE cost). PSUM accumulation is fp32.
  - Everything lives in [feature, token] layouts; no on-device transposes.
  - RMSNorm: per-chunk squares (Act) are reduced across partitions+chunks by
    an all-ones stationary matmul accumulating into a PSUM bank; rsqrt via
    Act-Sqrt + fast-approx reciprocal. Gammas and the 1/sqrt(QKD) scale are
    folded in (host folds gammas into up-weights, SCALE into the q norm).
  - RoPE: host duplicates the shared-key rope rows ([kr;kr], [rot;rot]) so
    on-device rope is elementwise; the roped dup key krd is gathered and
    split into [kre;0]/[0;kro] post-AG. q-side rotate_half is done on-chip
    with a +-1 permutation stationary matmul (no extra up-proj chunks).
  - Causal handling: strictly-upper score tiles are skipped; diagonal tiles
    run matmul/exp/AV only on the valid moving subrange, with one constant
    128x128 triangle mask added on the boundary block.
  - Softmax is unnormalized; per-tile denominators accumulate in a second
    PSUM bank via an all-ones stationary matmul (start/stop with the AV
    accumulation), then fast-approx reciprocal + scale at o eviction.
"""

import numpy as np
import ml_dtypes

import concourse.bass as bass
import concourse.mybir as mybir
import concourse.tile as tile
from concourse import bacc
from concourse.bass import ds, ts
from concourse.bass_utils import run_bass_kernel_spmd

F32 = mybir.dt.float32
BF16 = mybir.dt.bfloat16
AF = mybir.ActivationFunctionType

B, S, HID, NH = 2, 2048, 2048, 16
NOPE, ROPE, VD = 128, 64, 128
QKD = NOPE + ROPE
KVR, QR = 512, 1536
EPS = 1e-6
SCALE = QKD ** (-0.5)
P = 128
Q = 512                   # token quarter owned per core in phase A
NHC = HID // P            # 16 hidden chunks
NQC = QR // P             # 12 q-latent chunks
NKC = KVR // P            # 4 ckv chunks
NFC = 18                  # down-proj chunks: 12 q + 4 ckv + [kr;kr] + [rot;rot]
NDQ = 6                   # q_up output chunks: 4 nope + 2 rope pairs
NTT = S // Q              # 4 token tiles
NTC = S // P              # 16 token chunks
NEG = -1e30
RG = [[0, 1, 2, 3], [4, 5, 6, 7]]
DEBUG_DUMPS = False


def _emit(tc):
    nc = tc.nc
    hid_in = nc.dram_tensor("hid", [P, NHC, Q], BF16, kind="ExternalInput").ap()
    cosq_in = nc.dram_tensor("cosq", [P, Q], BF16, kind="ExternalInput").ap()
    sinq_in = nc.dram_tensor("sinq", [P, Q], BF16, kind="ExternalInput").ap()
    cos2_in = nc.dram_tensor("cos2", [P, S], BF16, kind="ExternalInput").ap()
    sin2_in = nc.dram_tensor("sin2", [P, S], BF16, kind="ExternalInput").ap()
    wd_in = nc.dram_tensor("wd", [NFC, P, NHC, P], BF16, kind="ExternalInput").ap()
    wqup_in = nc.dram_tensor("wqup", [P, NQC, NDQ * P], BF16, kind="ExternalInput").ap()
    wkup_in = nc.dram_tensor("wkup", [P, NKC, 512], BF16, kind="ExternalInput").ap()
    wvup_in = nc.dram_tensor("wvup", [P, NKC, 512], BF16, kind="ExternalInput").ap()
    wo_in = nc.dram_tensor("wo", [P, 4, HID], BF16, kind="ExternalInput").ap()
    rp_in = nc.dram_tensor("rperm", [P, P], BF16, kind="ExternalInput").ap()
    out_d = nc.dram_tensor("out", [S, HID], F32, kind="ExternalOutput").ap()
    if DEBUG_DUMPS:
        dbg_qT = nc.dram_tensor("dbg_qT", [P, NDQ, S], F32,
                                kind="ExternalOutput").ap()
        dbg_kt = nc.dram_tensor("dbg_kt", [P, 4, S], F32,
                                kind="ExternalOutput").ap()
        dbg_v = nc.dram_tensor("dbg_v", [P, NTC, 512], F32,
                               kind="ExternalOutput").ap()
        dbg_kre = nc.dram_tensor("dbg_kre", [P, S], F32,
                                 kind="ExternalOutput").ap()
        dbg_kro = nc.dram_tensor("dbg_kro", [P, S], F32,
                                 kind="ExternalOutput").ap()
        dbg_o = nc.dram_tensor("dbg_o", [P, 4, S], F32,
                               kind="ExternalOutput").ap()

    # AG bounce buffers (internal local DRAM) and gathered outputs
    bounce_kv = nc.dram_tensor("bkv", [5, P, Q], BF16, kind="Internal").ap()
    gath_kv = nc.dram_tensor("gkv", [4, 5, P, Q], BF16, kind="Internal").ap()
    bounce_q = nc.dram_tensor("bq", [NQC, P, Q], BF16, kind="Internal").ap()
    gath_q = nc.dram_tensor("gq", [4, NQC, P, Q], BF16, kind="Internal").ap()

    with tc.tile_pool(name="res", bufs=1) as res:
        # ---- constants ----
        ones_sb = res.tile([P, P], BF16, name="ones")
        nc.gpsimd.memset(ones_sb, 1.0)
        tri = res.tile([P, P], F32, name="tri")
        nc.gpsimd.memset(tri, 0.0)
        nc.gpsimd.affine_select(
            out=tri, in_=tri, pattern=[[1, P]],
            compare_op=mybir.AluOpType.is_ge, fill=NEG,
            base=0, channel_multiplier=-1,
        )
        eps_kv = res.tile([P, 1], F32, name="epskv")
        nc.vector.memset(eps_kv, EPS)
        eps_q = res.tile([P, 1], F32, name="epsq")
        nc.vector.memset(eps_q, EPS / (SCALE * SCALE))
        rp_sb = res.tile([P, P], BF16, name="rp")
        nc.sync.dma_start(rp_sb, rp_in)

        # ---- long-lived activation tiles ----
        qT = res.tile([P, NDQ, S], BF16, name="qT")
        kt_sb = res.tile([P, 4, S], BF16, name="kt")
        v_sb = res.tile([P, NTC, 512], BF16, name="v")
        kre = res.tile([P, S], BF16, name="kre")
        kro = res.tile([P, S], BF16, name="kro")
        o_sb = res.tile([P, 4, S], BF16, name="o")

        # ---- long-lived weights / tables ----
        wq_sb = res.tile([P, NQC, NDQ * P], BF16, name="wq")
        wk_sb = res.tile([P, NKC, 512], BF16, name="wk")
        wv_sb = res.tile([P, NKC, 512], BF16, name="wv")
        wo_sb = res.tile([P, 4, HID], BF16, name="wo")
        cos2 = res.tile([P, S], BF16, name="c2")
        sin2 = res.tile([P, S], BF16, name="s2")

        # ================ Phase A: quarter down-proj + norms + AGs ========
        with (
            tc.tile_pool(name="pa_hid", bufs=1) as ph,
            tc.tile_pool(name="pa_w", bufs=2) as pw,
            tc.tile_pool(name="pa_sq", bufs=2) as psq,
            tc.tile_pool(name="pa_ps", bufs=2, space="PSUM") as pps,
            tc.tile_pool(name="pa_nrm", bufs=1, space="PSUM") as pnrm,
        ):
            with nc.named_scope("phaseA"):
                hid_sb = ph.tile([P, NHC, Q], BF16)
                nc.sync.dma_start(hid_sb, hid_in)
                cosq = ph.tile([P, Q], BF16)
                sinq = ph.tile([P, Q], BF16)
                nc.sync.dma_start(cosq, cosq_in)
                nc.sync.dma_start(sinq, sinq_in)
                # prefetch long-lived weights (spare DMA queues during A)
                nc.sync.dma_start(wk_sb, wkup_in)
                nc.sync.dma_start(wv_sb, wvup_in)
                nc.sync.dma_start(wq_sb, wqup_in)
                nc.sync.dma_start(wo_sb, wo_in)
                nc.sync.dma_start(cos2, cos2_in)
                nc.sync.dma_start(sin2, sin2_in)

                ckv_sl = ph.tile([P, NKC, Q], BF16)  # normalized in place
                kr16 = ph.tile([P, Q], BF16)
                kr17 = ph.tile([P, Q], BF16)
                q_sl = ph.tile([P, NQC, Q], BF16)    # normalized in place
                ps_nkv = pnrm.tile([P, Q], F32, name="nkv")
                ps_nq = pnrm.tile([P, Q], F32, name="nq")

                def down_chunk(fc, dst, norm_ps, jn, jlast):
                    w_sb = pw.tile([P, NHC, P], BF16, name="wsl")
                    nc.sync.dma_start(w_sb, wd_in[fc])
                    ps = pps.tile([P, Q], F32, name="aps")
                    for hc in range(NHC):
                        nc.tensor.matmul(
                            ps, w_sb[:, hc, :], hid_sb[:, hc, :],
                            start=(hc == 0), stop=(hc == NHC - 1),
                        )
                    if norm_ps is not None:
                        sq = psq.tile([P, Q], BF16, name="sq")
                        nc.scalar.square(sq, ps)
                        nc.tensor.matmul(norm_ps, ones_sb, sq,
                                         start=(jn == 0), stop=jlast)
                    nc.vector.tensor_copy(dst, ps)

                # ckv chunks (fc 12..15) + rope chunks (16,17) first
                for j in range(NKC):
                    down_chunk(12 + j, ckv_sl[:, j, :], ps_nkv, j, j == NKC - 1)
                down_chunk(16, kr16, None, 0, False)
                down_chunk(17, kr17, None, 0, False)
                # rkv = 1/sqrt(mean(ckv^2)+eps)
                rkv = psq.tile([P, Q], F32, name="rkv")
                nc.scalar.activation(rkv, ps_nkv, AF.Sqrt, bias=eps_kv,
                                     scale=1.0 / KVR)
                nc.vector.reciprocal_approx_fast(out=rkv, in_=rkv)
                for j in range(NKC):
                    nc.vector.tensor_mul(ckv_sl[:, j, :], ckv_sl[:, j, :], rkv)
                    nc.sync.dma_start(bounce_kv[j], ckv_sl[:, j, :])
                # krd = kr*cos + rot(kr)*sin (duplicated layout)
                t1 = psq.tile([P, Q], BF16, name="krt1")
                nc.vector.tensor_mul(t1, kr16, cosq)
                nc.vector.tensor_mul(kr17, kr17, sinq)
                nc.vector.tensor_add(kr16, t1, kr17)
                nc.sync.dma_start(bounce_kv[4], kr16)
                nc.gpsimd.collective_compute(
                    "AllGather", mybir.AluOpType.bypass, replica_groups=RG,
                    ins=[bounce_kv], outs=[gath_kv],
                )

                # q-latent chunks (fc 0..11)
                for j in range(NQC):
                    down_chunk(j, q_sl[:, j, :], ps_nq, j, j == NQC - 1)
                rq = psq.tile([P, Q], F32, name="rq")
                nc.scalar.activation(rq, ps_nq, AF.Sqrt, bias=eps_q,
                                     scale=1.0 / (QR * SCALE * SCALE))
                nc.vector.reciprocal_approx_fast(out=rq, in_=rq)
                for j in range(NQC):
                    nc.vector.tensor_mul(q_sl[:, j, :], q_sl[:, j, :], rq)
                    nc.sync.dma_start(bounce_q[j], q_sl[:, j, :])
                nc.gpsimd.collective_compute(
                    "AllGather", mybir.AluOpType.bypass, replica_groups=RG,
                    ins=[bounce_q], outs=[gath_q],
                )

        # ================ Phase B: kv_up (kt, v) + kre/kro ================
        with (
            tc.tile_pool(name="pb_in", bufs=1) as pbi,
            tc.tile_pool(name="pb_ps", bufs=3, space="PSUM") as pps2,
        ):
            with nc.named_scope("phaseB"):
                ckv_g = pbi.tile([P, NKC, S], BF16)
                for fc in range(NKC):
                    for r in range(4):
                        nc.sync.dma_start(ckv_g[:, fc, ts(r, Q)], gath_kv[r, fc])
                # kre = [krd;0], kro = [0;krd] (krd halves are duplicates)
                nc.gpsimd.memset(kre, 0.0)
                nc.gpsimd.memset(kro, 0.0)
                for r in range(4):
                    nc.sync.dma_start(kre[0:64, ts(r, Q)], gath_kv[r, 4][0:64])
                    nc.sync.dma_start(kro[64:128, ts(r, Q)], gath_kv[r, 4][64:128])
                # kt: per head, k_nope^T over all tokens
                for d in range(4):
                    for tt in range(NTT):
                        ps = pps2.tile([P, Q], F32, name="bps")
                        for fc in range(NKC):
                            nc.tensor.matmul(
                                ps, wk_sb[:, fc, ds(d * P, P)],
                                ckv_g[:, fc, ts(tt, Q)],
                                start=(fc == 0), stop=(fc == NKC - 1),
                            )
                        if (d + tt) % 2 == 0:
                            nc.vector.tensor_copy(kt_sb[:, d, ts(tt, Q)], ps)
                        else:
                            nc.scalar.copy(kt_sb[:, d, ts(tt, Q)], ps)
                # V: [token, 4*VD] per 128-token chunk
                for tch in range(NTC):
                    ps = pps2.tile([P, 512], F32, name="bpsv")
                    for fc in range(NKC):
                        nc.tensor.matmul(
                            ps, ckv_g[:, fc, ds(tch * P, P)], wv_sb[:, fc, :],
                            start=(fc == 0), stop=(fc == NKC - 1),
                        )
                    if tch % 2 == 0:
                        nc.vector.tensor_copy(v_sb[:, tch, :], ps)
                    else:
                        nc.scalar.copy(v_sb[:, tch, :], ps)

        # ============ Phases C/D/F interleaved per token tile =============
        with (
            tc.tile_pool(name="pc_sl", bufs=2) as pcs,
            tc.tile_pool(name="pc_t", bufs=3) as pct,
            tc.tile_pool(name="pc_ps", bufs=2, space="PSUM") as pcp,
            tc.tile_pool(name="pd_sc", bufs=2, space="PSUM") as pdsc,
            tc.tile_pool(name="pd_et", bufs=3) as pde,
            tc.tile_pool(name="pd_o", bufs=2, space="PSUM") as pdo,
            tc.tile_pool(name="pd_d", bufs=2, space="PSUM") as pdd,
            tc.tile_pool(name="pf_r", bufs=2) as pfr,
        ):
            for i in range(NTT):
                # ---------------- C(i): q_up for token tile i -------------
                with nc.named_scope(f"phaseC{i}"):
                    slab = pcs.tile([P, NQC, Q], BF16, name="qslab")
                    for fc in range(NQC):
                        nc.sync.dma_start(slab[:, fc, :], gath_q[i, fc])
                    for d in range(4):
                        ps = pcp.tile([P, Q], F32, name="cps")
                        for fc in range(NQC):
                            nc.tensor.matmul(
                                ps, wq_sb[:, fc, ds(d * P, P)], slab[:, fc, :],
                                start=(fc == 0), stop=(fc == NQC - 1),
                            )
                        if d % 2 == 0:
                            nc.vector.tensor_copy(qT[:, d, ts(i, Q)], ps)
                        else:
                            nc.scalar.copy(qT[:, d, ts(i, Q)], ps)
                    for pr in range(2):
                        ps = pcp.tile([P, Q], F32, name="cps")
                        for fc in range(NQC):
                            nc.tensor.matmul(
                                ps, wq_sb[:, fc, ds((4 + pr) * P, P)],
                                slab[:, fc, :],
                                start=(fc == 0), stop=(fc == NQC - 1),
                            )
                        pair = pct.tile([P, Q], BF16, name="pair")
                        nc.vector.tensor_copy(pair, ps)
                        psr = pdd.tile([P, Q], F32, name="psd")
                        nc.tensor.matmul(psr, rp_sb, pair, start=True, stop=True)
                        t1 = pct.tile([P, Q], BF16, name="ct1")
                        nc.vector.tensor_mul(t1, pair, cos2[:, ts(i, Q)])
                        t2 = pct.tile([P, Q], BF16, name="ct2")
                        nc.vector.tensor_mul(t2, psr, sin2[:, ts(i, Q)])
                        nc.vector.tensor_add(qT[:, 4 + pr, ts(i, Q)], t1, t2)

                # ---------------- D(h, i): attention ----------------------
                with nc.named_scope(f"phaseD{i}"):
                    for h in range(4):
                        krop = kre if h % 2 == 0 else kro
                        jmax = 4 * i + 3
                        ps_o = pdo.tile([P, Q], F32, name="pso")
                        ps_d = pdd.tile([P, Q], F32, name="psd")
                        for jc in range(jmax + 1):
                            r = jc - 4 * i
                            lo = P * r if r > 0 else 0
                            n = Q - lo
                            psc = pdsc.tile([P, Q], F32, name="psc")
                            nc.tensor.matmul(
                                psc[:, ds(lo, n)], kt_sb[:, h, ds(jc * P, P)],
                                qT[:, h, ds(i * Q + lo, n)],
                                start=True, stop=False)
                            nc.tensor.matmul(
                                psc[:, ds(lo, n)], krop[:, ds(jc * P, P)],
                                qT[:, 4 + h // 2, ds(i * Q + lo, n)],
                                start=False, stop=True)
                            if r >= 0:
                                nc.vector.tensor_add(
                                    psc[:, ds(lo, P)], psc[:, ds(lo, P)], tri)
                            et = pde.tile([P, Q], BF16, name="et")
                            nc.scalar.activation(et[:, ds(lo, n)],
                                                 psc[:, ds(lo, n)], AF.Exp)
                            nc.tensor.matmul(
                                ps_o[:, ds(lo, n)],
                                v_sb[:, jc, ds(h * P, P)], et[:, ds(lo, n)],
                                start=(jc == 0), stop=(jc == jmax))
                            nc.tensor.matmul(
                                ps_d[:, ds(lo, n)], ones_sb, et[:, ds(lo, n)],
                                start=(jc == 0), stop=(jc == jmax))
                        ar = pct.tile([P, Q], F32, name="dar")
                        nc.vector.reciprocal_approx_fast(out=ar, in_=ps_d)
                        nc.vector.tensor_mul(o_sb[:, h, ts(i, Q)], ps_o, ar)

                # ---------------- F(i): o_proj partial --------------------
                with nc.named_scope(f"phaseF{i}"):
                    for tl in range(4):
                        tch = 4 * i + tl
                        orow = pfr.tile([P, HID], F32, name="orow")
                        for ct in range(4):
                            ps = pdd.tile([P, Q], F32, name="psd")
                            for hh in range(4):
                                nc.tensor.matmul(
                                    ps, o_sb[:, hh, ds(tch * P, P)],
                                    wo_sb[:, hh, ts(ct, Q)],
                                    start=(hh == 0), stop=(hh == 3),
                                )
                            if ct % 2 == 0:
                                nc.vector.tensor_copy(orow[:, ts(ct, Q)], ps)
                            else:
                                nc.scalar.copy(orow[:, ts(ct, Q)], ps)
                        nc.sync.dma_start(out_d[ds(tch * P, P), :], orow)

            if DEBUG_DUMPS:
                with tc.tile_pool(name="dbg", bufs=2) as pdbg:
                    def dump(dst, src, n):
                        for j in range(n):
                            t = pdbg.tile([P, S], F32, name="dbgt")
                            nc.vector.tensor_copy(t[:, 0:src.shape[-1]],
                                                  src[:, j, :] if n > 1 else src)
                            nc.sync.dma_start(
                                dst[:, j, :] if n > 1 else dst,
                                t[:, 0:src.shape[-1]])
                    dump(dbg_qT, qT, NDQ)
                    dump(dbg_kt, kt_sb, 4)
                    dump(dbg_v, v_sb, NTC)
                    dump(dbg_kre, kre, 1)
                    dump(dbg_kro, kro, 1)
                    dump(dbg_o, o_sb, 4)


_NC_CACHE = None


def _build_nc():
    global _NC_CACHE
    if _NC_CACHE is None:
        nc = bacc.Bacc("TRN2", target_bir_lowering=False, debug=False,
                       num_devices=8)
        with tile.TileContext(nc) as tc:
            _emit(tc)
        nc.compile()
        _NC_CACHE = nc
    return _NC_CACHE


def _shard_inputs(hidden_states, cos, sin, Wq_down, q_gamma, Wq_up,
                  Wkv_down, kv_gamma, Wkv_up, Wo):
    f32 = np.float32
    bf16 = ml_dtypes.bfloat16
    hid = np.asarray(hidden_states, dtype=f32)
    cos = np.asarray(cos, dtype=f32)
    sin = np.asarray(sin, dtype=f32)
    Wqd = np.asarray(Wq_down, dtype=f32)
    Wkd = np.asarray(Wkv_down, dtype=f32)
    qg = np.asarray(q_gamma, dtype=f32)
    kvg = np.asarray(kv_gamma, dtype=f32)
    Wqu = np.asarray(Wq_up, dtype=f32) * qg[None, :]
    Wku = np.asarray(Wkv_up, dtype=f32) * kvg[None, :]
    Wo = np.asarray(Wo, dtype=f32)

    # fused down-proj weight: 12 q + 4 ckv + [kr;kr] + [rot;rot]
    WqdT = Wqd.T                                   # [HID, QR]
    WckvT = Wkd[:KVR].T                            # [HID, KVR]
    kr = Wkd[KVR:].T                               # [HID, 64]
    krot = np.concatenate([-kr[:, 32:], kr[:, :32]], 1)
    WdT = np.concatenate(
        [WqdT, WckvT,
         np.concatenate([kr, kr], 1),
         np.concatenate([krot, krot], 1)], 1)      # [HID, 2304]
    wd = np.ascontiguousarray(
        WdT.reshape(NHC, P, NFC, P).transpose(2, 1, 0, 3)).astype(bf16)

    # rotate_half permutation (+-1) for the q rope pairs
    R = np.zeros((P, P), dtype=f32)
    for blk in (0, 64):
        for m in range(32):
            R[blk + m + 32, blk + m] = -1.0
            R[blk + m, blk + m + 32] = 1.0
    R = R.astype(bf16)

    per_batch = []
    for b in range(B):
        cT = cos[b].T                              # [64, S]
        sT = sin[b].T
        cos2 = np.ascontiguousarray(np.concatenate([cT, cT], 0)).astype(bf16)
        sin2 = np.ascontiguousarray(np.concatenate([sT, sT], 0)).astype(bf16)
        quarters = []
        for tq in range(4):
            hq = hid[b, tq * Q:(tq + 1) * Q]       # [512, HID]
            h_sw = np.ascontiguousarray(
                hq.T.reshape(NHC, P, Q).transpose(1, 0, 2)).astype(bf16)
            quarters.append(h_sw)
        per_batch.append((quarters, cos2, sin2))

    per_group = []
    for g in range(4):
        bn, br = [], []
        for hl in range(4):
            h = 4 * g + hl
            blk = Wqu[h * QKD:(h + 1) * QKD]       # [192, QR]
            bn.append(blk[:NOPE])
            br.append(blk[NOPE:])
        cols = bn + [np.concatenate([br[0], br[1]], 0),
                     np.concatenate([br[2], br[3]], 0)]
        WquT = np.concatenate(cols, 0).T           # [QR, 768]
        wqup = np.ascontiguousarray(
            WquT.reshape(NQC, P, NDQ * P).transpose(1, 0, 2)).astype(bf16)
        kb, vb = [], []
        for hl in range(4):
            h = 4 * g + hl
            blk = Wku[h * (NOPE + VD):(h + 1) * (NOPE + VD)]
            kb.append(blk[:NOPE])
            vb.append(blk[NOPE:])
        WkuT = np.concatenate(kb, 0).T             # [KVR, 512]
        WvuT = np.concatenate(vb, 0).T
        wkup = np.ascontiguousarray(
            WkuT.reshape(NKC, P, 512).transpose(1, 0, 2)).astype(bf16)
        wvup = np.ascontiguousarray(
            WvuT.reshape(NKC, P, 512).transpose(1, 0, 2)).astype(bf16)
        WoT = Wo[:, g * 512:(g + 1) * 512].T       # [512, HID]
        wo = np.ascontiguousarray(
            WoT.reshape(4, P, HID).transpose(1, 0, 2)).astype(bf16)
        per_group.append((wqup, wkup, wvup, wo))

    in_maps = []
    for c in range(8):
        b, g = c // 4, c % 4
        quarters, cos2, sin2 = per_batch[b]
        wqup, wkup, wvup, wo = per_group[g]
        tq = c % 4
        cqT = cos[b, tq * Q:(tq + 1) * Q].T        # [64, 512]
        sqT = sin[b, tq * Q:(tq + 1) * Q].T
        in_maps.append({
            "hid": quarters[tq],
            "cosq": np.ascontiguousarray(
                np.concatenate([cqT, cqT], 0)).astype(bf16),
            "sinq": np.ascontiguousarray(
                np.concatenate([sqT, sqT], 0)).astype(bf16),
            "cos2": cos2, "sin2": sin2,
            "wd": wd, "wqup": wqup, "wkup": wkup, "wvup": wvup, "wo": wo,
            "rperm": R,
        })
    return in_maps


def kernel(hidden_states, cos, sin, Wq_down, q_gamma, Wq_up,
           Wkv_down, kv_gamma, Wkv_up, Wo, _trace=False):
    nc = _build_nc()
    in_maps = _shard_inputs(hidden_states, cos, sin, Wq_down, q_gamma, Wq_up,
                            Wkv_down, kv_gamma, Wkv_up, Wo)
    res = run_bass_kernel_spmd(nc, in_maps, core_ids=list(range(8)),
                               trace=_trace)
    out = np.zeros((B, S, HID), dtype=np.float32)
    for c in range(8):
        out[c // 4] += res.results[c]["out"]
    if _trace:
        kernel.last_results = res
    return out


# revision 35
# speedup vs baseline: 1.8285x; 1.0900x over previous
"""MLA (Multi-head Latent Attention) Bass/Tile kernel for 8 Trainium2 NeuronCores.

Problem: nn_MultiHeadLatentAttention_81707457839331
  B=2, S=2048, HID=2048, NH=16 heads, NOPE=128, ROPE=64, VD=128, QKD=192,
  KVR=512, QR=1536, fp32 in/out.

Sharding (single NEFF, SPMD on 8 cores, collectives inside):
  core c -> batch b = c//4, head group g = c%4 (4 heads), token quarter
  tq = c%4 (512 tokens) for the down-projection.
  Phase A computes the fused down-projection ONLY for the core's token
  quarter, RMS-normalizes q-latent / c_kv and ropes the shared key locally,
  then two AllGathers (ckv+krd first, q-latent second) over the 4-core batch
  group replicate the full-sequence latents. kv_up/q_up/attention/o_proj are
  head-sharded as before; each core emits a partial o_proj [S, HID] and the
  host sums the 4 partials per batch.

Numerics/layout:
  - All matmul inputs are bf16 (PE rate is 1 cycle/row either way; bf16
    halves DMA, SBUF and DVE cost). PSUM accumulation is fp32.
  - Everything lives in [feature, token] layouts; no on-device transposes.
  - RMSNorm: per-chunk squares (Act) are reduced across partitions+chunks by
    an all-ones stationary matmul accumulating into a PSUM bank; rsqrt via
    Act-Sqrt + fast-approx reciprocal. Gammas and the 1/sqrt(QKD) scale are
    folded in (host folds gammas into up-weights, SCALE into the q norm).
  - RoPE: host duplicates the shared-key rope rows ([kr;kr], [rot;rot]) so
    on-device rope is elementwise; the roped dup key krd is gathered and
    split into [kre;0]/[0;kro] post-AG. q-side rotate_half is done on-chip
    with a +-1 permutation stationary matmul (no extra up-proj chunks).
  - Causal handling: strictly-upper score tiles are skipped; diagonal tiles
    run matmul/exp/AV only on the valid moving subrange, with one constant
    128x128 triangle mask added on the boundary block.
  - Softmax is unnormalized; per-tile denominators accumulate in a second
    PSUM bank via an all-ones stationary matmul (start/stop with the AV
    accumulation), then fast-approx reciprocal + scale at o eviction.
"""

import numpy as np
import ml_dtypes

import concourse.bass as bass
import concourse.mybir as mybir
import concourse.tile as tile
from concourse import bacc
from concourse.bass import ds, ts
from concourse.bass_utils import run_bass_kernel_spmd

F32 = mybir.dt.float32
BF16 = mybir.dt.bfloat16
AF = mybir.ActivationFunctionType

B, S, HID, NH = 2, 2048, 2048, 16
NOPE, ROPE, VD = 128, 64, 128
QKD = NOPE + ROPE
KVR, QR = 512, 1536
EPS = 1e-6
SCALE = QKD ** (-0.5)
P = 128
Q = 512                   # token quarter owned per core in phase A
NHC = HID // P            # 16 hidden chunks
NQC = QR // P             # 12 q-latent chunks
NKC = KVR // P            # 4 ckv chunks
NFC = 18                  # down-proj chunks: 12 q + 4 ckv + [kr;kr] + [rot;rot]
NDQ = 6                   # q_up output chunks: 4 nope + 2 rope pairs
NTT = S // Q              # 4 token tiles
NTC = S // P              # 16 token chunks
NEG = -1e30
RG = [[0, 1, 2, 3], [4, 5, 6, 7]]
DEBUG_DUMPS = False


def _emit(tc):
    nc = tc.nc
    hid_in = nc.dram_tensor("hid", [P, NHC, Q], BF16, kind="ExternalInput").ap()
    cosq_in = nc.dram_tensor("cosq", [P, Q], BF16, kind="ExternalInput").ap()
    sinq_in = nc.dram_tensor("sinq", [P, Q], BF16, kind="ExternalInput").ap()
    cos2_in = nc.dram_tensor("cos2", [P, S], BF16, kind="ExternalInput").ap()
    sin2_in = nc.dram_tensor("sin2", [P, S], BF16, kind="ExternalInput").ap()
    wd_in = nc.dram_tensor("wd", [NFC, P, NHC, P], BF16, kind="ExternalInput").ap()
    wqup_in = nc.dram_tensor("wqup", [P, NQC, NDQ * P], BF16, kind="ExternalInput").ap()
    wkup_in = nc.dram_tensor("wkup", [P, NKC, 512], BF16, kind="ExternalInput").ap()
    wvup_in = nc.dram_tensor("wvup", [P, NKC, 512], BF16, kind="ExternalInput").ap()
    wo_in = nc.dram_tensor("wo", [P, 4, HID], BF16, kind="ExternalInput").ap()
    rp_in = nc.dram_tensor("rperm", [P, P], BF16, kind="ExternalInput").ap()
    out_d = nc.dram_tensor("out", [S, HID], F32, kind="ExternalOutput").ap()
    if DEBUG_DUMPS:
        dbg_qT = nc.dram_tensor("dbg_qT", [P, NDQ, S], F32,
                                kind="ExternalOutput").ap()
        dbg_kt = nc.dram_tensor("dbg_kt", [P, 4, S], F32,
                                kind="ExternalOutput").ap()
        dbg_v = nc.dram_tensor("dbg_v", [P, NTC, 512], F32,
                               kind="ExternalOutput").ap()
        dbg_kre = nc.dram_tensor("dbg_kre", [P, S], F32,
                                 kind="ExternalOutput").ap()
        dbg_kro = nc.dram_tensor("dbg_kro", [P, S], F32,
                                 kind="ExternalOutput").ap()
        dbg_o = nc.dram_tensor("dbg_o", [P, 4, S], F32,
                               kind="ExternalOutput").ap()

    # AG bounce buffers (internal local DRAM) and gathered outputs, flattened
    # [blocks*128, 512]; 4-core batch-group AllGather, rank block = quarter.
    bounce_kv = nc.dram_tensor("bkv", [5 * P, Q], BF16, kind="Internal").ap()
    gath_kv = nc.dram_tensor("gkv", [4 * 5 * P, Q], BF16, kind="Internal").ap()
    bounce_q = nc.dram_tensor("bq", [NQC * P, Q], BF16, kind="Internal").ap()
    gath_q = nc.dram_tensor("gq", [4 * NQC * P, Q], BF16,
                            kind="Internal").ap()

    with tc.tile_pool(name="res", bufs=1) as res:
        # ---- constants ----
        ones_sb = res.tile([P, P], BF16, name="ones")
        nc.gpsimd.memset(ones_sb, 1.0)
        tri = res.tile([P, P], F32, name="tri")
        nc.gpsimd.memset(tri, 0.0)
        nc.gpsimd.affine_select(
            out=tri, in_=tri, pattern=[[1, P]],
            compare_op=mybir.AluOpType.is_ge, fill=NEG,
            base=0, channel_multiplier=-1,
        )
        eps_kv = res.tile([P, 1], F32, name="epskv")
        nc.vector.memset(eps_kv, EPS)
        eps_q = res.tile([P, 1], F32, name="epsq")
        nc.vector.memset(eps_q, EPS / (SCALE * SCALE))
        rp_sb = res.tile([P, P], BF16, name="rp")
        nc.sync.dma_start(rp_sb, rp_in)


        # ---- long-lived activation tiles ----
        qT = res.tile([P, NDQ, S], BF16, name="qT")
        kt_sb = res.tile([P, 4, S], BF16, name="kt")
        v_sb = res.tile([P, NTC, 512], BF16, name="v")
        kre = res.tile([P, S], BF16, name="kre")
        kro = res.tile([P, S], BF16, name="kro")
        o_sb = res.tile([P, 4, S], BF16, name="o")

        # ---- long-lived weights / tables ----
        wq_sb = res.tile([P, NQC, NDQ * P], BF16, name="wq")
        wk_sb = res.tile([P, NKC, 512], BF16, name="wk")
        wv_sb = res.tile([P, NKC, 512], BF16, name="wv")
        wo_sb = res.tile([P, 4, HID], BF16, name="wo")
        cos2 = res.tile([P, S], BF16, name="c2")
        sin2 = res.tile([P, S], BF16, name="s2")

        # ================ Phase A: quarter down-proj + norms + AGs ========
        with (
            tc.tile_pool(name="pa_hid", bufs=1) as ph,
            tc.tile_pool(name="pa_w", bufs=2) as pw,
            tc.tile_pool(name="pa_sq", bufs=2) as psq,
            tc.tile_pool(name="pa_ps", bufs=2, space="PSUM") as pps,
            tc.tile_pool(name="pa_nrm", bufs=1, space="PSUM") as pnrm,
        ):
            with nc.named_scope("phaseA"):
                hid_sb = ph.tile([P, NHC, Q], BF16)
                for hq in range(4):
                    nc.sync.dma_start(hid_sb[:, ds(4 * hq, 4), :],
                                      hid_in[:, ds(4 * hq, 4), :])
                cosq = ph.tile([P, Q], BF16)
                sinq = ph.tile([P, Q], BF16)
                nc.sync.dma_start(cosq, cosq_in)
                nc.sync.dma_start(sinq, sinq_in)

                ckv_sl = ph.tile([P, NKC, Q], BF16)  # normalized in place
                kr16 = ph.tile([P, Q], BF16)
                kr17 = ph.tile([P, Q], BF16)
                q_sl = ph.tile([P, NQC, Q], BF16)    # normalized in place
                ps_nkv = pnrm.tile([P, Q], F32, name="nkv")
                ps_nq = pnrm.tile([P, Q], F32, name="nq")

                def down_chunk(fc, dst, norm_ps, jn, jlast):
                    w_sb = pw.tile([P, NHC, P], BF16, name="wsl")
                    nc.sync.dma_start(w_sb, wd_in[fc])
                    ps = pps.tile([P, Q], F32, name="aps")
                    for hc in range(NHC):
                        nc.tensor.matmul(
                            ps, w_sb[:, hc, :], hid_sb[:, hc, :],
                            start=(hc == 0), stop=(hc == NHC - 1),
                        )
                    if norm_ps is not None:
                        sq = psq.tile([P, Q], BF16, name="sq")
                        nc.scalar.square(sq, ps)
                        nc.tensor.matmul(norm_ps, ones_sb, sq,
                                         start=(jn == 0), stop=jlast)
                    nc.vector.tensor_copy(dst, ps)

                # ckv chunks (fc 12..15) + rope chunks (16,17) first
                for j in range(NKC):
                    down_chunk(12 + j, ckv_sl[:, j, :], ps_nkv, j, j == NKC - 1)
                # prefetch long-lived weights (behind the first wd loads)
                nc.sync.dma_start(wk_sb, wkup_in)
                nc.sync.dma_start(wv_sb, wvup_in)
                nc.sync.dma_start(wq_sb, wqup_in)
                nc.sync.dma_start(wo_sb, wo_in)
                nc.sync.dma_start(cos2, cos2_in)
                nc.sync.dma_start(sin2, sin2_in)
                down_chunk(16, kr16, None, 0, False)
                down_chunk(17, kr17, None, 0, False)
                # rkv = 1/sqrt(mean(ckv^2)+eps)
                rkv = psq.tile([P, Q], F32, name="rkv")
                nc.scalar.activation(rkv, ps_nkv, AF.Sqrt, bias=eps_kv,
                                     scale=1.0 / KVR)
                nc.vector.reciprocal_approx_fast(out=rkv, in_=rkv)
                for j in range(NKC):
                    nc.vector.tensor_mul(ckv_sl[:, j, :], ckv_sl[:, j, :], rkv)
                    nc.sync.dma_start(bounce_kv[ds(j * P, P)], ckv_sl[:, j, :])
                # krd = kr*cos + rot(kr)*sin (duplicated layout)
                t1 = psq.tile([P, Q], BF16, name="krt1")
                nc.vector.tensor_mul(t1, kr16, cosq)
                nc.vector.tensor_mul(kr17, kr17, sinq)
                nc.vector.tensor_add(kr16, t1, kr17)
                nc.sync.dma_start(bounce_kv[ds(4 * P, P)], kr16)
                nc.gpsimd.collective_compute(
                    "AllGather", mybir.AluOpType.bypass, replica_groups=RG,
                    ins=[bounce_kv], outs=[gath_kv],
                )

                # q-latent chunks (fc 0..11)
                for j in range(NQC):
                    down_chunk(j, q_sl[:, j, :], ps_nq, j, j == NQC - 1)
                rq = psq.tile([P, Q], F32, name="rq")
                nc.scalar.activation(rq, ps_nq, AF.Sqrt, bias=eps_q,
                                     scale=1.0 / (QR * SCALE * SCALE))
                nc.vector.reciprocal_approx_fast(out=rq, in_=rq)
                for j in range(NQC):
                    nc.vector.tensor_mul(q_sl[:, j, :], q_sl[:, j, :], rq)
                    nc.sync.dma_start(bounce_q[ds(j * P, P)], q_sl[:, j, :])
                nc.gpsimd.collective_compute(
                    "AllGather", mybir.AluOpType.bypass, replica_groups=RG,
                    ins=[bounce_q], outs=[gath_q],
                )

        # ================ Phase B: kv_up (kt, v) + kre/kro ================
        with (
            tc.tile_pool(name="pb_in", bufs=1) as pbi,
            tc.tile_pool(name="pb_ps", bufs=3, space="PSUM") as pps2,
        ):
            with nc.named_scope("phaseB"):
                ckv_g = pbi.tile([P, NKC, S], BF16)
                for fc in range(NKC):
                    for r in range(4):
                        nc.sync.dma_start(
                            ckv_g[:, fc, ts(r, Q)],
                            gath_kv[ds((r * 5 + fc) * P, P)])
                # kre = [krd;0], kro = [0;krd] (krd halves are duplicates)
                nc.gpsimd.memset(kre, 0.0)
                nc.gpsimd.memset(kro, 0.0)
                for r in range(4):
                    nc.sync.dma_start(
                        kre[0:64, ts(r, Q)],
                        gath_kv[ds((r * 5 + 4) * P, 64)])
                    nc.sync.dma_start(
                        kro[64:128, ts(r, Q)],
                        gath_kv[ds((r * 5 + 4) * P + 64, 64)])
                # kt: per head, k_nope^T over all tokens
                for d in range(4):
                    for tt in range(NTT):
                        ps = pps2.tile([P, Q], F32, name="bps")
                        for fc in range(NKC):
                            nc.tensor.matmul(
                                ps, wk_sb[:, fc, ds(d * P, P)],
                                ckv_g[:, fc, ts(tt, Q)],
                                start=(fc == 0), stop=(fc == NKC - 1),
                            )
                        if (d + tt) % 2 == 0:
                            nc.vector.tensor_copy(kt_sb[:, d, ts(tt, Q)], ps)
                        else:
                            nc.scalar.copy(kt_sb[:, d, ts(tt, Q)], ps)
                # V: [token, 4*VD] per 128-token chunk
                for tch in range(NTC):
                    ps = pps2.tile([P, 512], F32, name="bpsv")
                    for fc in range(NKC):
                        nc.tensor.matmul(
                            ps, ckv_g[:, fc, ds(tch * P, P)], wv_sb[:, fc, :],
                            start=(fc == 0), stop=(fc == NKC - 1),
                        )
                    if tch % 2 == 0:
                        nc.vector.tensor_copy(v_sb[:, tch, :], ps)
                    else:
                        nc.scalar.copy(v_sb[:, tch, :], ps)

        # ============ Phases C/D/F interleaved per token tile =============
        with (
            tc.tile_pool(name="pc_sl", bufs=2) as pcs,
            tc.tile_pool(name="pc_t", bufs=3) as pct,
            tc.tile_pool(name="pd_sc", bufs=4, space="PSUM") as pdsc,
            tc.tile_pool(name="pd_et", bufs=4) as pde,
            tc.tile_pool(name="pd_o", bufs=2, space="PSUM") as pdo,
            tc.tile_pool(name="pd_d", bufs=2, space="PSUM") as pdd,
            tc.tile_pool(name="pf_r", bufs=2) as pfr,
        ):
            for i in range(NTT):
                # ---------------- C(i): q_up for token tile i -------------
                with nc.named_scope(f"phaseC{i}"):
                    slab = pcs.tile([P, NQC, Q], BF16, name="qslab")
                    for fc in range(NQC):
                        nc.sync.dma_start(
                            slab[:, fc, :],
                            gath_q[ds((i * NQC + fc) * P, P)])
                    for d in range(4):
                        ps = pdsc.tile([P, Q], F32, name="psc")
                        for fc in range(NQC):
                            nc.tensor.matmul(
                                ps, wq_sb[:, fc, ds(d * P, P)], slab[:, fc, :],
                                start=(fc == 0), stop=(fc == NQC - 1),
                            )
                        if d % 2 == 0:
                            nc.vector.tensor_copy(qT[:, d, ts(i, Q)], ps)
                        else:
                            nc.scalar.copy(qT[:, d, ts(i, Q)], ps)
                    for pr in range(2):
                        ps = pdsc.tile([P, Q], F32, name="psc")
                        for fc in range(NQC):
                            nc.tensor.matmul(
                                ps, wq_sb[:, fc, ds((4 + pr) * P, P)],
                                slab[:, fc, :],
                                start=(fc == 0), stop=(fc == NQC - 1),
                            )
                        pair = pct.tile([P, Q], BF16, name="pair")
                        nc.vector.tensor_copy(pair, ps)
                        psr = pdd.tile([P, Q], F32, name="psd")
                        nc.tensor.matmul(psr, rp_sb, pair, start=True, stop=True)
                        t1 = pct.tile([P, Q], BF16, name="ct1")
                        nc.vector.tensor_mul(t1, pair, cos2[:, ts(i, Q)])
                        t2 = pct.tile([P, Q], BF16, name="ct2")
                        nc.vector.tensor_mul(t2, psr, sin2[:, ts(i, Q)])
                        nc.vector.tensor_add(qT[:, 4 + pr, ts(i, Q)], t1, t2)

                # ------ D(i): attention, two heads interleaved on PE ------
                with nc.named_scope(f"phaseD{i}"):
                    jmax = 4 * i + 3
                    for hp in range(2):
                        heads = (2 * hp, 2 * hp + 1)
                        ps_o = {h: pdo.tile([P, Q], F32, name="pso")
                                for h in heads}
                        ps_d = {h: pdd.tile([P, Q], F32, name="psd")
                                for h in heads}
                        for jc in range(jmax + 1):
                            r = jc - 4 * i
                            lo = P * r if r > 0 else 0
                            n = Q - lo
                            ets = {}
                            for h in heads:
                                krop = kre if h % 2 == 0 else kro
                                psc = pdsc.tile([P, Q], F32, name="psc")
                                nc.tensor.matmul(
                                    psc[:, ds(lo, n)],
                                    kt_sb[:, h, ds(jc * P, P)],
                                    qT[:, h, ds(i * Q + lo, n)],
                                    start=True, stop=False)
                                nc.tensor.matmul(
                                    psc[:, ds(lo, n)], krop[:, ds(jc * P, P)],
                                    qT[:, 4 + h // 2, ds(i * Q + lo, n)],
                                    start=False, stop=True)
                                if r >= 0:
                                    nc.vector.tensor_add(
                                        psc[:, ds(lo, P)], psc[:, ds(lo, P)],
                                        tri)
                                et = pde.tile([P, Q], BF16, name="et")
                                nc.scalar.activation(et[:, ds(lo, n)],
                                                     psc[:, ds(lo, n)], AF.Exp)
                                ets[h] = et
                            for h in heads:
                                et = ets[h]
                                nc.tensor.matmul(
                                    ps_o[h][:, ds(lo, n)],
                                    v_sb[:, jc, ds(h * P, P)],
                                    et[:, ds(lo, n)],
                                    start=(jc == 0), stop=(jc == jmax))
                                nc.tensor.matmul(
                                    ps_d[h][:, ds(lo, n)], ones_sb,
                                    et[:, ds(lo, n)],
                                    start=(jc == 0), stop=(jc == jmax))
                        for h in heads:
                            ar = pct.tile([P, Q], F32, name="dar")
                            nc.vector.reciprocal_approx_fast(
                                out=ar, in_=ps_d[h])
                            nc.vector.tensor_mul(o_sb[:, h, ts(i, Q)],
                                                 ps_o[h], ar)

                # ---------------- F(i): o_proj partial --------------------
                with nc.named_scope(f"phaseF{i}"):
                    for tl in range(4):
                        tch = 4 * i + tl
                        orow = pfr.tile([P, HID], F32, name="orow")
                        for ct in range(4):
                            ps = pdd.tile([P, Q], F32, name="psd")
                            for hh in range(4):
                                nc.tensor.matmul(
                                    ps, o_sb[:, hh, ds(tch * P, P)],
                                    wo_sb[:, hh, ts(ct, Q)],
                                    start=(hh == 0), stop=(hh == 3),
                                )
                            if ct % 2 == 0:
                                nc.vector.tensor_copy(orow[:, ts(ct, Q)], ps)
                            else:
                                nc.scalar.copy(orow[:, ts(ct, Q)], ps)
                        nc.sync.dma_start(out_d[ds(tch * P, P), :], orow)

            if DEBUG_DUMPS:
                with tc.tile_pool(name="dbg", bufs=2) as pdbg:
                    def dump(dst, src, n):
                        for j in range(n):
                            t = pdbg.tile([P, S], F32, name="dbgt")
                            nc.vector.tensor_copy(t[:, 0:src.shape[-1]],
                                                  src[:, j, :] if n > 1 else src)
                            nc.sync.dma_start(
                                dst[:, j, :] if n > 1 else dst,
                                t[:, 0:src.shape[-1]])
                    dump(dbg_qT, qT, NDQ)
                    dump(dbg_kt, kt_sb, 4)
                    dump(dbg_v, v_sb, NTC)
                    dump(dbg_kre, kre, 1)
                    dump(dbg_kro, kro, 1)
                    dump(dbg_o, o_sb, 4)


_NC_CACHE = None


def _build_nc():
    global _NC_CACHE
    if _NC_CACHE is None:
        nc = bacc.Bacc("TRN2", target_bir_lowering=False, debug=False,
                       num_devices=8)
        with tile.TileContext(nc) as tc:
            _emit(tc)
        nc.compile()
        _NC_CACHE = nc
    return _NC_CACHE


def _shard_inputs(hidden_states, cos, sin, Wq_down, q_gamma, Wq_up,
                  Wkv_down, kv_gamma, Wkv_up, Wo):
    f32 = np.float32
    bf16 = ml_dtypes.bfloat16
    hid = np.asarray(hidden_states, dtype=f32)
    cos = np.asarray(cos, dtype=f32)
    sin = np.asarray(sin, dtype=f32)
    Wqd = np.asarray(Wq_down, dtype=f32)
    Wkd = np.asarray(Wkv_down, dtype=f32)
    qg = np.asarray(q_gamma, dtype=f32)
    kvg = np.asarray(kv_gamma, dtype=f32)
    Wqu = np.asarray(Wq_up, dtype=f32) * qg[None, :]
    Wku = np.asarray(Wkv_up, dtype=f32) * kvg[None, :]
    Wo = np.asarray(Wo, dtype=f32)

    # fused down-proj weight: 12 q + 4 ckv + [kr;kr] + [rot;rot]
    WqdT = Wqd.T                                   # [HID, QR]
    WckvT = Wkd[:KVR].T                            # [HID, KVR]
    kr = Wkd[KVR:].T                               # [HID, 64]
    krot = np.concatenate([-kr[:, 32:], kr[:, :32]], 1)
    WdT = np.concatenate(
        [WqdT, WckvT,
         np.concatenate([kr, kr], 1),
         np.concatenate([krot, krot], 1)], 1)      # [HID, 2304]
    wd = np.ascontiguousarray(
        WdT.reshape(NHC, P, NFC, P).transpose(2, 1, 0, 3)).astype(bf16)

    # rotate_half permutation (+-1) for the q rope pairs
    R = np.zeros((P, P), dtype=f32)
    for blk in (0, 64):
        for m in range(32):
            R[blk + m + 32, blk + m] = -1.0
            R[blk + m, blk + m + 32] = 1.0
    R = R.astype(bf16)

    per_batch = []
    for b in range(B):
        cT = cos[b].T                              # [64, S]
        sT = sin[b].T
        cos2 = np.ascontiguousarray(np.concatenate([cT, cT], 0)).astype(bf16)
        sin2 = np.ascontiguousarray(np.concatenate([sT, sT], 0)).astype(bf16)
        quarters = []
        for tq in range(4):
            hq = hid[b, tq * Q:(tq + 1) * Q]       # [512, HID]
            h_sw = np.ascontiguousarray(
                hq.T.reshape(NHC, P, Q).transpose(1, 0, 2)).astype(bf16)
            quarters.append(h_sw)
        per_batch.append((quarters, cos2, sin2))

    per_group = []
    for g in range(4):
        bn, br = [], []
        for hl in range(4):
            h = 4 * g + hl
            blk = Wqu[h * QKD:(h + 1) * QKD]       # [192, QR]
            bn.append(blk[:NOPE])
            br.append(blk[NOPE:])
        cols = bn + [np.concatenate([br[0], br[1]], 0),
                     np.concatenate([br[2], br[3]], 0)]
        WquT = np.concatenate(cols, 0).T           # [QR, 768]
        wqup = np.ascontiguousarray(
            WquT.reshape(NQC, P, NDQ * P).transpose(1, 0, 2)).astype(bf16)
        kb, vb = [], []
        for hl in range(4):
            h = 4 * g + hl
            blk = Wku[h * (NOPE + VD):(h + 1) * (NOPE + VD)]
            kb.append(blk[:NOPE])
            vb.append(blk[NOPE:])
        WkuT = np.concatenate(kb, 0).T             # [KVR, 512]
        WvuT = np.concatenate(vb, 0).T
        wkup = np.ascontiguousarray(
            WkuT.reshape(NKC, P, 512).transpose(1, 0, 2)).astype(bf16)
        wvup = np.ascontiguousarray(
            WvuT.reshape(NKC, P, 512).transpose(1, 0, 2)).astype(bf16)
        WoT = Wo[:, g * 512:(g + 1) * 512].T       # [512, HID]
        wo = np.ascontiguousarray(
            WoT.reshape(4, P, HID).transpose(1, 0, 2)).astype(bf16)
        per_group.append((wqup, wkup, wvup, wo))

    in_maps = []
    for c in range(8):
        b, g = c // 4, c % 4
        quarters, cos2, sin2 = per_batch[b]
        wqup, wkup, wvup, wo = per_group[g]
        tq = c % 4
        cqT = cos[b, tq * Q:(tq + 1) * Q].T        # [64, 512]
        sqT = sin[b, tq * Q:(tq + 1) * Q].T
        in_maps.append({
            "hid": quarters[tq],
            "cosq": np.ascontiguousarray(
                np.concatenate([cqT, cqT], 0)).astype(bf16),
            "sinq": np.ascontiguousarray(
                np.concatenate([sqT, sqT], 0)).astype(bf16),
            "cos2": cos2, "sin2": sin2,
            "wd": wd, "wqup": wqup, "wkup": wkup, "wvup": wvup, "wo": wo,
            "rperm": R,
        })
    return in_maps


def kernel(hidden_states, cos, sin, Wq_down, q_gamma, Wq_up,
           Wkv_down, kv_gamma, Wkv_up, Wo, _trace=False):
    nc = _build_nc()
    in_maps = _shard_inputs(hidden_states, cos, sin, Wq_down, q_gamma, Wq_up,
                            Wkv_down, kv_gamma, Wkv_up, Wo)
    res = run_bass_kernel_spmd(nc, in_maps, core_ids=list(range(8)),
                               trace=_trace)
    out = np.zeros((B, S, HID), dtype=np.float32)
    for c in range(8):
        out[c // 4] += res.results[c]["out"]
    if _trace:
        kernel.last_results = res
    return out
